# revision 3
# baseline (speedup 1.0000x reference)
"""Deformable Conv2d — bf16 pipeline, corner-difference lerp, batched gather.

Sharding: (batch=4) x (H halves=2) -> 8 cores; core computes out[b,:,h0:h0+64,:].

Per-core pipeline, software-pipelined in 4 blocks of 16 rows:
  1. offset conv (18ch) in bf16 via PE over 1px-padded image (PSUM fp32).
  2. PE-transpose offsets to point-major [128pts, 18] (fp32).
  3. DVE fp32 coord math -> fy/fx fracs, int32 gather row-index into a
     2px-padded ABCD image in DRAM; fx/fy/fx*fy duplicated x2 and cast bf16
     (dup makes the broadcast AP's last dim packed => DVE fast modes).
  4. One indirect DMA per GCH groups: offsets [128, GCH*9] -> gt
     [128, GCH*9*256] bf16.  Per descriptor 512B = (A,B,C,D)x64ch where
     A=x00, B=x01-x00, C=x10-x00, D=x11-x10-x01+x00 (host precomputed).
  5. DVE 6-op lerp via scalar_tensor_tensor (4x_2p mode):
     s = A + fx*B + fy*C + fxy*D   [128, 9*64] bf16.
  6. PE-transpose s (bf16) -> PSUM, 2 ACT copies -> SBUF bf16, 5 bf16
     matmuls (K=576), ACT bias-add fp32, chunked DMA out.
"""

import sys
for p in ("/opt/trn_rl_repo",):
    if p not in sys.path:
        sys.path.insert(0, p)

import numpy as np
import ml_dtypes

import concourse.bacc as bacc
import concourse.mybir as mybir
import concourse.tile as tile
from concourse.bass import IndirectOffsetOnAxis
from concourse.bass_utils import run_bass_kernel_spmd

F32 = mybir.dt.float32
BF16 = mybir.dt.bfloat16
I32 = mybir.dt.int32
AL = mybir.AluOpType
AF = mybir.ActivationFunctionType
BF = ml_dtypes.bfloat16

B, C, H, W = 4, 64, 128, 128
K, KK = 3, 9
O = 64
OC = 2 * KK                  # offset channels (18)
HL = H // 2                  # local rows per core (64)
NPT = HL * W                 # local points per core (8192)
NG = NPT // 128              # groups of 128 points (=64); group g == local row g
W2 = W + 2                   # offset-conv padded width (130)
H2 = HL + 2                  # offset-conv padded local rows (66)
W4 = W + 4                   # ABCD image width (132)
H4 = H + 4                   # ABCD image height (132)
MAGIC = float(3 * 2 ** 22)
GCH = 4                      # groups per gather instruction
GBLK = KK * 256              # gathered bf16 elems per point per group (2304)
RB = 16                      # rows (=groups) per pipeline block
NB = NG // RB                # blocks (4)


def build_program(repeat=1):
    nc = bacc.Bacc("TRN2", target_bir_lowering=False, debug=False)

    xp = nc.dram_tensor("xp", [C, H2 * W2], BF16, kind="ExternalInput")
    xcl = nc.dram_tensor("xcl", [H4 * W4, 256], BF16, kind="ExternalInput")
    wofft = nc.dram_tensor("wofft", [C, KK * OC], BF16, kind="ExternalInput")
    woffb = nc.dram_tensor("woffb", [OC, 1], F32, kind="ExternalInput")
    wmain = nc.dram_tensor("wmain", [128, 5 * O], BF16, kind="ExternalInput")
    wb = nc.dram_tensor("wb", [O, 1], F32, kind="ExternalInput")
    basey = nc.dram_tensor("basey", [128, NG * KK], F32, kind="ExternalInput")
    basex = nc.dram_tensor("basex", [128, NG * KK], F32, kind="ExternalInput")
    ident = nc.dram_tensor("ident", [128, 128], BF16, kind="ExternalInput")
    identf = nc.dram_tensor("identf", [OC, OC], F32, kind="ExternalInput")
    out = nc.dram_tensor("out", [O, NPT], F32, kind="ExternalOutput")

    with tile.TileContext(nc) as tc:
        with (
            tc.tile_pool(name="cst", bufs=1) as cst,
            tc.tile_pool(name="keep", bufs=1) as keep,
            tc.tile_pool(name="blkp", bufs=2) as blkp,
            tc.tile_pool(name="gat", bufs=2) as gat,
            tc.tile_pool(name="lrp", bufs=2) as lrp,
            tc.tile_pool(name="psA", bufs=2, space="PSUM") as psA,
            tc.tile_pool(name="psT", bufs=2, space="PSUM") as psT,
            tc.tile_pool(name="psO", bufs=2, space="PSUM") as psO,
        ):
            ident_t = cst.tile([128, 128], BF16, tag="ident")
            nc.sync.dma_start(out=ident_t[:], in_=ident[:])
            wofft_t = cst.tile([C, KK * OC], BF16, tag="wofft")
            nc.sync.dma_start(out=wofft_t[:], in_=wofft[:])
            woffb_t = cst.tile([OC, 1], F32, tag="woffb")
            nc.sync.dma_start(out=woffb_t[:], in_=woffb[:])
            wmain_t = cst.tile([128, 5 * O], BF16, tag="wmain")
            nc.sync.dma_start(out=wmain_t[:], in_=wmain[:])
            wb_t = cst.tile([O, 1], F32, tag="wb")
            nc.sync.dma_start(out=wb_t[:], in_=wb[:])
            basey_t = cst.tile([128, NG * KK], F32, tag="basey")
            nc.sync.dma_start(out=basey_t[:], in_=basey[:])
            basex_t = cst.tile([128, NG * KK], F32, tag="basex")
            nc.sync.dma_start(out=basex_t[:], in_=basex[:])
            identf_t = cst.tile([OC, OC], F32, tag="identf")
            nc.sync.dma_start(out=identf_t[:], in_=identf[:])

            STT = nc.vector.scalar_tensor_tensor

            for _rep in range(repeat):
                NW = NG * KK
                fx2 = keep.tile([128, NW * 2], BF16, tag="fx2")
                fy2 = keep.tile([128, NW * 2], BF16, tag="fy2")
                fz2 = keep.tile([128, NW * 2], BF16, tag="fz2")
                idx = keep.tile([128, NW], I32, tag="idx")
                xp_t = keep.tile([C, H2 * W2], BF16, tag="xp")
                nc.sync.dma_start(out=xp_t[:], in_=xp[:])
                xp3 = xp_t[:].rearrange("c (h w) -> c h w", h=H2)
                out_sb = keep.tile([O, NPT], F32, tag="osb")

                for blk in range(NB):
                    g0 = blk * RB          # first group (=row) of block
                    BW = RB * KK           # coord width per block (144)

                    # ---- offset conv rows g0..g0+RB: OFF[18, RB*W] ----
                    off_t = blkp.tile([OC, RB * W], F32, tag="off")
                    RPC = 4
                    for r0 in range(0, RB, RPC):
                        ps = psA.tile([OC, RPC * W], F32, tag="psA")
                        for kk in range(KK):
                            ki, kj = kk // K, kk % K
                            rhs = xp3[:, g0 + r0 + ki:g0 + r0 + ki + RPC,
                                      kj:kj + W]
                            nc.tensor.matmul(
                                out=ps[:], lhsT=wofft_t[:, kk * OC:(kk + 1) * OC],
                                rhs=rhs, start=(kk == 0), stop=(kk == KK - 1))
                        nc.scalar.activation(
                            out=off_t[:, r0 * W:(r0 + RPC) * W], in_=ps[:],
                            func=AF.Identity, bias=woffb_t[:, 0:1], scale=1.0)

                    # ---- transpose block offsets to point-major ----
                    offT = blkp.tile([128, RB * OC], F32, tag="offT")
                    for g in range(RB):
                        ps = psA.tile([128, OC], F32, tag="psA")
                        nc.tensor.transpose(
                            out=ps[:], in_=off_t[:, g * 128:(g + 1) * 128],
                            identity=identf_t[:])
                        nc.scalar.copy(out=offT[:, g * OC:(g + 1) * OC], in_=ps[:])

                    # ---- coord math (fp32, block-wide) ----
                    o4 = offT[:].rearrange("p (g k t) -> p g k t", g=RB, k=KK)
                    dy = o4[:, :, :, 0]
                    dx = o4[:, :, :, 1]

                    ys = blkp.tile([128, BW], F32, tag="ys")
                    xs = blkp.tile([128, BW], F32, tag="xs")
                    rr = blkp.tile([128, BW], F32, tag="rr")
                    mm = blkp.tile([128, BW], F32, tag="mm")
                    y0 = blkp.tile([128, BW], F32, tag="y0")
                    x0 = blkp.tile([128, BW], F32, tag="x0")
                    fy = blkp.tile([128, BW], F32, tag="fyf")
                    fx = blkp.tile([128, BW], F32, tag="fxf")
                    fz = blkp.tile([128, BW], F32, tag="fzf")

                    ys3 = ys[:].rearrange("p (g k) -> p g k", g=RB)
                    xs3 = xs[:].rearrange("p (g k) -> p g k", g=RB)
                    sl = slice(g0 * KK, (g0 + RB) * KK)
                    by3 = basey_t[:, sl].rearrange("p (g k) -> p g k", g=RB)
                    bx3 = basex_t[:, sl].rearrange("p (g k) -> p g k", g=RB)
                    nc.vector.tensor_tensor(out=ys3, in0=dy, in1=by3, op=AL.add)
                    nc.vector.tensor_tensor(out=xs3, in0=dx, in1=bx3, op=AL.add)

                    def floorv(src, dst, frac):
                        nc.vector.tensor_scalar(
                            out=rr[:], in0=src[:], scalar1=MAGIC, scalar2=MAGIC,
                            op0=AL.add, op1=AL.subtract)
                        nc.vector.tensor_tensor(out=mm[:], in0=rr[:],
                                                in1=src[:], op=AL.is_gt)
                        nc.vector.tensor_tensor(out=dst[:], in0=rr[:],
                                                in1=mm[:], op=AL.subtract)
                        nc.vector.tensor_tensor(out=frac[:], in0=src[:],
                                                in1=dst[:], op=AL.subtract)

                    floorv(ys, y0, fy)
                    floorv(xs, x0, fx)
                    nc.vector.tensor_tensor(out=fz[:], in0=fx[:], in1=fy[:],
                                            op=AL.mult)
                    # clamp ints to [-2, 128]
                    nc.vector.tensor_scalar(out=rr[:], in0=y0[:], scalar1=-2.0,
                                            scalar2=float(H), op0=AL.max,
                                            op1=AL.min)
                    nc.vector.tensor_scalar(out=mm[:], in0=x0[:], scalar1=-2.0,
                                            scalar2=float(W), op0=AL.max,
                                            op1=AL.min)
                    # row index into ABCD image: (y0c+2)*W4 + (x0c+2)
                    nc.vector.scalar_tensor_tensor(
                        out=ys[:], in0=rr[:], scalar=float(W4), in1=mm[:],
                        op0=AL.mult, op1=AL.add)
                    nc.vector.tensor_scalar(
                        out=idx[:, sl], in0=ys[:], scalar1=float(2 * W4 + 2),
                        scalar2=None, op0=AL.add)
                    # duplicate fracs x2, cast to bf16
                    for src, dst in ((fx, fx2), (fy, fy2), (fz, fz2)):
                        d3 = dst[:, g0 * KK * 2:(g0 + RB) * KK * 2] \
                            .rearrange("p (w d) -> p w d", d=2)
                        nc.vector.tensor_copy(out=d3[:, :, 0], in_=src[:])
                        nc.vector.tensor_copy(out=d3[:, :, 1], in_=src[:])

                    # ---- main loop for this block ----
                    for c0 in range(g0, g0 + RB, GCH):
                        gt = gat.tile([128, GCH * GBLK], BF16, tag="G")
                        for gs in range(GCH):
                            for kk in range(KK):
                                col = (c0 + gs) * KK + kk
                                nc.gpsimd.indirect_dma_start(
                                    out=gt[:, (gs * KK + kk) * 256:
                                           (gs * KK + kk + 1) * 256],
                                    out_offset=None, in_=xcl[:],
                                    in_offset=IndirectOffsetOnAxis(
                                        ap=idx[:, col:col + 1], axis=0))
                        for gs in range(GCH):
                            g = c0 + gs
                            g5 = gt[:, gs * GBLK:(gs + 1) * GBLK].rearrange(
                                "p (k q cp d) -> p k q cp d", k=KK, q=4, cp=32)
                            va = g5[:, :, 0]           # [128, 9, 32, 2]
                            vb = g5[:, :, 1]
                            vc = g5[:, :, 2]
                            vd = g5[:, :, 3]

                            def wop(t2):
                                return t2[:, g * KK * 2:(g + 1) * KK * 2] \
                                    .rearrange("p (k d) -> p k d", d=2) \
                                    .unsqueeze(2).to_broadcast([128, KK, 32, 2])

                            s_ = lrp.tile([128, KK * C], BF16, tag="s")
                            t_ = lrp.tile([128, KK * C], BF16, tag="t")
                            TT = nc.vector.tensor_tensor
                            s4 = s_[:].rearrange("p (k cp d) -> p k cp d",
                                                 k=KK, d=2)
                            t4 = t_[:].rearrange("p (k cp d) -> p k cp d",
                                                 k=KK, d=2)
                            TT(out=t4, in0=vb, in1=wop(fx2), op=AL.mult)
                            TT(out=s4, in0=va, in1=t4, op=AL.add)
                            TT(out=t4, in0=vc, in1=wop(fy2), op=AL.mult)
                            TT(out=s4, in0=s4, in1=t4, op=AL.add)
                            TT(out=t4, in0=vd, in1=wop(fz2), op=AL.mult)
                            TT(out=s4, in0=s4, in1=t4, op=AL.add)

                            # transpose s -> channel-major chunks
                            ps4 = psT.tile([128, 640], BF16, tag="psT4")
                            for j in range(4):
                                nc.tensor.transpose(
                                    out=ps4[:, j * 128:(j + 1) * 128],
                                    in_=s_[:, j * 128:(j + 1) * 128],
                                    identity=ident_t[:])
                            nc.tensor.transpose(
                                out=ps4[:64, 512:640], in_=s_[:, 512:576],
                                identity=ident_t[:])
                            st = lrp.tile([128, 640], BF16, tag="st")
                            nc.scalar.copy(out=st[:, 0:512], in_=ps4[:, 0:512])
                            nc.scalar.copy(out=st[:64, 512:640],
                                           in_=ps4[:64, 512:640])

                            po = psO.tile([O, 128], F32, tag="psO")
                            for j in range(4):
                                nc.tensor.matmul(
                                    out=po[:], lhsT=wmain_t[:, j * O:(j + 1) * O],
                                    rhs=st[:, j * 128:(j + 1) * 128],
                                    start=(j == 0), stop=False)
                            nc.tensor.matmul(
                                out=po[:], lhsT=wmain_t[:64, 4 * O:5 * O],
                                rhs=st[:64, 512:640], start=False, stop=True)
                            nc.scalar.activation(
                                out=out_sb[:, g * 128:(g + 1) * 128], in_=po[:],
                                func=AF.Identity, bias=wb_t[:, 0:1], scale=1.0)

                    nc.sync.dma_start(
                        out=out[:, g0 * W:(g0 + RB) * W],
                        in_=out_sb[:, g0 * W:(g0 + RB) * W])

    nc.compile()
    return nc


_NC_CACHE = None


def _get_nc():
    global _NC_CACHE
    if _NC_CACHE is None:
        _NC_CACHE = build_program()
    return _NC_CACHE


def make_core_inputs(x, weight, bias, offset_w, offset_b):
    x = np.asarray(x, np.float32)
    weight = np.asarray(weight, np.float32)
    bias = np.asarray(bias, np.float32)
    offset_w = np.asarray(offset_w, np.float32)
    offset_b = np.asarray(offset_b, np.float32)

    # offset-conv image: 1px pad
    xp_full = np.pad(x, ((0, 0), (0, 0), (1, 1), (1, 1)))

    # ABCD image: 2px pad + 1 extra bottom/right for the +1 taps
    xe = np.pad(x, ((0, 0), (0, 0), (2, 3), (2, 3)))
    a = xe[:, :, :H4, :W4]
    bb = xe[:, :, :H4, 1:W4 + 1] - a
    cc = xe[:, :, 1:H4 + 1, :W4] - a
    dd = (xe[:, :, 1:H4 + 1, 1:W4 + 1] - xe[:, :, 1:H4 + 1, :W4]) - bb
    zz = np.empty((B, H4, W4, 256), BF)
    zz[..., 0:64] = a.transpose(0, 2, 3, 1)
    zz[..., 64:128] = bb.transpose(0, 2, 3, 1)
    zz[..., 128:192] = cc.transpose(0, 2, 3, 1)
    zz[..., 192:256] = dd.transpose(0, 2, 3, 1)

    # offset conv weights [c, kk*18]
    wofft = np.ascontiguousarray(
        offset_w.reshape(OC, C, KK).transpose(1, 2, 0)).reshape(C, KK * OC)
    woffb = offset_b.reshape(OC, 1)
    # main conv weights, (kk, ch)-major rows: wflat[kk*64+c, o] = weight[o,c,kk]
    wflat = weight.reshape(O, C, KK).transpose(2, 1, 0).reshape(KK * C, O)
    wmain = np.zeros((128, 5 * O), np.float32)
    for j in range(5):
        rows = min(128, KK * C - j * 128)
        wmain[:rows, j * O:(j + 1) * O] = wflat[j * 128:j * 128 + rows]
    wb = bias.reshape(O, 1)
    identm = np.eye(128, dtype=np.float32)

    p = np.arange(128, dtype=np.float32)
    g = np.arange(NG, dtype=np.float32)
    kki = (np.arange(KK) // K).astype(np.float32)
    kkj = (np.arange(KK) % K).astype(np.float32)
    basex = (p[:, None, None] - 1.0 + kkj[None, None, :]) + 0.0 * g[None, :, None]
    basex = np.ascontiguousarray(
        np.broadcast_to(basex, (128, NG, KK)), np.float32).reshape(128, NG * KK)

    in_maps = []
    for core in range(8):
        b, h0 = core // 2, (core % 2) * HL
        by = np.broadcast_to(
            (h0 + g)[None, :, None] - 1.0 + kki[None, None, :], (128, NG, KK))
        in_maps.append({
            "xp": np.ascontiguousarray(
                xp_full[b, :, h0:h0 + H2, :]).astype(BF).reshape(C, H2 * W2),
            "xcl": np.ascontiguousarray(zz[b]).reshape(H4 * W4, 256),
            "wofft": wofft.astype(BF), "woffb": woffb,
            "wmain": wmain.astype(BF), "wb": wb,
            "basey": np.ascontiguousarray(by, np.float32).reshape(128, NG * KK),
            "basex": basex,
            "ident": identm.astype(BF),
            "identf": identm[:OC, :OC].copy(),
        })
    return in_maps


def kernel(x, weight, bias, offset_w, offset_b):
    nc = _get_nc()
    in_maps = make_core_inputs(x, weight, bias, offset_w, offset_b)
    res = run_bass_kernel_spmd(nc, in_maps, list(range(8)))
    out_full = np.empty((B, O, H, W), np.float32)
    for core in range(8):
        b, h0 = core // 2, (core % 2) * HL
        out_full[b, :, h0:h0 + HL, :] = res.results[core]["out"].reshape(O, HL, W)
    return out_full
